# revision 1
# baseline (speedup 1.0000x reference)
"""Multi-head attention Trainium2 kernel (B=4, S=2048, E=1024, H=16).

Sharding: 8 cores = 4 batch groups x 2-way head tensor-parallel.
Core c handles batch b=c//2 and heads [g*8, g*8+8) with g=c%2.
Each core computes its partial output projection; a 2-way ReduceScatter
pair-sums the partials in 8 row chunks, so core c ends with interleaved
128-row slabs of batch b's final output. The host assembles the full
[4,2048,1024] result.

Device layout notes:
- x arrives pre-transposed from the host as xT[e,s] so every matmul
  contracts over the SBUF partition dim.
- Q,K are produced transposed (QT/KT [d,s]) with head pairs stacked on the
  128 partitions; scores are computed transposed (scoresT[k,q]) so the exp
  output PT[k,q] feeds the P@V matmul directly as the moving operand.
- exp runs on 1024-wide strips (two query blocks) to amortize the ACT
  engine's ~293-cycle per-op overhead; P@V runs one key-block behind the
  scores stream so the PE never waits on the exp it just scheduled.
- V gets a ones-column (65th) so the P@V matmul also emits the softmax
  denominator row for free; normalization uses a fast DVE reciprocal.
- All matmuls run in bf16 (fp32 PSUM accumulate).
"""

import os
import sys

import numpy as np

for _p in ("/opt/trn_rl_repo", "/root/.axon_site/_ro/trn_rl_repo"):
    if os.path.isdir(_p) and _p not in sys.path:
        sys.path.append(_p)

import ml_dtypes  # noqa: E402
from concourse import bacc, mybir, tile  # noqa: E402
from concourse.bass_utils import run_bass_kernel_spmd  # noqa: E402

B, S, E, H, DH = 4, 2048, 1024, 16, 64
N_CORES = 8
TP = 2  # head-parallel factor within a batch
H_LOC = H // TP  # 8 heads per core
EI_LOC = H_LOC * DH  # 512 local rows of the concat dim
N_SB = S // 128  # 16 token blocks
N_EC = E // 128  # 8 contraction chunks
N_QB = S // 512  # 4 query blocks
N_KB = S // 128  # 16 key blocks
N_HP = H_LOC // 2  # 4 head pairs
# ReduceScatter chunk schedule: (row0, nrows, emit_after_qb)
RS_CHUNKS = [(0, 1024, 1), (1024, 1024, 3)]

BF = mybir.dt.bfloat16
F32 = mybir.dt.float32
EXP = mybir.ActivationFunctionType.Exp
MULT = mybir.AluOpType.mult

_CACHE = {}


def _build():
    nc = bacc.Bacc("TRN2", target_bir_lowering=False, debug=False,
                   num_devices=N_CORES)

    xT_in = nc.declare_dram_parameter("xT", [E, S], BF, isOutput=False)
    wq_in = nc.declare_dram_parameter("wq", [E, EI_LOC], BF, isOutput=False)
    wk_in = nc.declare_dram_parameter("wk", [E, EI_LOC], BF, isOutput=False)
    wv_in = nc.declare_dram_parameter("wv", [E, EI_LOC], BF, isOutput=False)
    woT_in = nc.declare_dram_parameter("woT", [EI_LOC, E], BF, isOutput=False)
    bob_in = nc.declare_dram_parameter("bob", [128, E], F32, isOutput=False)
    y_out = nc.declare_dram_parameter("y", [S // TP, E], F32, isOutput=True)

    y_part = nc.dram_tensor("y_part", [S, E], F32)
    y_chunks = [nc.dram_tensor(f"y_chunk{i}", [n // 2, E], F32)
                for i, (_, n, _) in enumerate(RS_CHUNKS)]

    inv_sqrt_dh = 1.0 / float(np.sqrt(DH))

    with tile.TileContext(nc) as tc:
        with (
            tc.tile_pool(name="const", bufs=1) as constp,
            tc.tile_pool(name="persist", bufs=1) as persist,
        ):
            # ---- input DMAs (xT first: projections are the startup
            # critical path) ----
            xTp = tc.alloc_tile_pool(name="xTp", bufs=1)
            xT = [xTp.tile([128, S], BF, tag=f"xT{ec}", name=f"xT{ec}")
                  for ec in range(N_EC)]
            cs0 = slice(0, 512)
            for ec in range(N_EC):
                nc.sync.dma_start(
                    xT[ec][:, cs0], xT_in[ec * 128:(ec + 1) * 128, cs0])
            wq_t, wk_t, wv_t = [], [], []
            for ec in range(N_EC):
                t = constp.tile([128, EI_LOC], BF, tag=f"wv{ec}",
                                name=f"wv{ec}")
                nc.sync.dma_start(t[:], wv_in[ec * 128:(ec + 1) * 128, :])
                wv_t.append(t)
            for q in range(1, 4):
                cs = slice(q * 512, (q + 1) * 512)
                for ec in range(N_EC):
                    nc.sync.dma_start(
                        xT[ec][:, cs], xT_in[ec * 128:(ec + 1) * 128, cs])
            for ec in range(N_EC):
                for lst, src, nm in ((wq_t, wq_in, "wq"), (wk_t, wk_in, "wk")):
                    t = constp.tile([128, EI_LOC], BF, tag=f"{nm}{ec}",
                                    name=f"{nm}{ec}")
                    nc.sync.dma_start(t[:], src[ec * 128:(ec + 1) * 128, :])
                    lst.append(t)
            woT_t = []
            for c in range(4):
                t = constp.tile([128, E], BF, tag=f"woT{c}", name=f"woT{c}")
                nc.sync.dma_start(t[:], woT_in[c * 128:(c + 1) * 128, :])
                woT_t.append(t)
            bob = constp.tile([128, E], F32, tag="bob")
            nc.sync.dma_start(bob[:], bob_in[:])

            # Per-head QT/KT tiles zero-padded to K=128 so the scores
            # matmuls stream the full PE array (keeps the activity monitor
            # out of its throttled state; data rows 0-63, zeros 64-127).
            QT = [persist.tile([128, S], BF, tag=f"QT{h}", name=f"QT{h}")
                  for h in range(H_LOC)]
            KT = [persist.tile([128, S], BF, tag=f"KT{h}", name=f"KT{h}")
                  for h in range(H_LOC)]
            for h in range(H_LOC):
                nc.vector.memset(QT[h][64:128, :], 0.0)
                nc.vector.memset(KT[h][64:128, :], 0.0)
            V = [persist.tile([128, H_LOC, DH + 1], BF, tag=f"V{s}",
                              name=f"V{s}") for s in range(N_SB)]
            CT = [persist.tile([128, S], BF, tag=f"CT{c}", name=f"CT{c}")
                  for c in range(4)]

            # ---- phase A: projections (own PSUM pool, closed before
            # attention) ----
            with tc.tile_pool(name="projps", bufs=4, space="PSUM") as projps:
                # V projection (natural layout [s, h*d]) + ones column
                for sb in range(N_SB):
                    ps = projps.tile([128, EI_LOC], F32, tag="projp",
                                     name="vps")
                    for ec in range(N_EC):
                        nc.tensor.matmul(
                            ps[:], xT[ec][:, sb * 128:(sb + 1) * 128],
                            wv_t[ec][:], start=(ec == 0),
                            stop=(ec == N_EC - 1))
                    nc.vector.tensor_copy(V[sb][:, :, 0:DH], ps[:])
                    nc.vector.memset(V[sb][:, :, DH], 1.0)

                # Q/K transposed projections, head pairs on partitions
                for hp in range(N_HP):
                    for qb in range(N_QB):
                        for dst, w in ((QT, wq_t), (KT, wk_t)):
                            ps = projps.tile([128, 512], F32, tag="projp",
                                             name="qkps")
                            for ec in range(N_EC):
                                nc.tensor.matmul(
                                    ps[:],
                                    w[ec][:, hp * 128:(hp + 1) * 128],
                                    xT[ec][:, qb * 512:(qb + 1) * 512],
                                    start=(ec == 0), stop=(ec == N_EC - 1))
                            cols = slice(qb * 512, (qb + 1) * 512)
                            nc.vector.tensor_copy(
                                dst[2 * hp][0:64, cols], ps[0:64, :])
                            nc.vector.tensor_copy(
                                dst[2 * hp + 1][0:64, cols], ps[64:128, :])

            xTp.release()

            # ---- phase B: attention ----
            with (
                tc.tile_pool(name="scps", bufs=2, space="PSUM") as scps,
                tc.tile_pool(name="pvps", bufs=4, space="PSUM") as pvps,
                tc.tile_pool(name="ptp", bufs=6) as ptp,
                tc.tile_pool(name="smallp", bufs=3) as smallp,
                tc.tile_pool(name="youtp", bufs=4) as youtp,
            ):
                _attention(nc, tc, scps, pvps, ptp, smallp, youtp,
                           QT, KT, V, CT, woT_t, bob,
                           y_part, y_chunks, y_out, inv_sqrt_dh)

    nc.finalize()
    return nc


def _attention(nc, tc, scps, pvps, ptp, smallp, youtp, QT, KT, V, CT, woT_t,
               bob, y_part, y_chunks, y_out, inv_sqrt_dh):
    if True:  # keep indentation shallow
        if True:
            for qp in range(N_QB // 2):  # query pair-blocks of 1024
                for h in range(H_LOC):
                    hp, hh = h // 2, h % 2
                    rows = slice(hh * 64, (hh + 1) * 64)
                    pv0 = pvps.tile([DH + 1, 512], F32, tag="pv", name="pv0")
                    pv1 = pvps.tile([DH + 1, 512], F32, tag="pv", name="pv1")
                    prev_pt = None
                    for kb in range(N_KB):
                        sp = scps.tile([128, 1024], F32, tag="sc", name="sc")
                        for half in range(2):
                            q5 = slice((2 * qp + half) * 512,
                                       (2 * qp + half + 1) * 512)
                            nc.tensor.matmul(
                                sp[:, half * 512:(half + 1) * 512],
                                KT[h][:, kb * 128:(kb + 1) * 128],
                                QT[h][:, q5])
                        pt = ptp.tile([128, 1024], BF, tag="pt", name="pt")
                        nc.scalar.activation(pt[:], sp[:], EXP,
                                             scale=inv_sqrt_dh)
                        if prev_pt is not None:
                            pkb = kb - 1
                            nc.tensor.matmul(
                                pv0[:], V[pkb][:, h, :], prev_pt[:, 0:512],
                                start=(pkb == 0), stop=False)
                            nc.tensor.matmul(
                                pv1[:], V[pkb][:, h, :], prev_pt[:, 512:1024],
                                start=(pkb == 0), stop=False)
                        prev_pt = pt
                    nc.tensor.matmul(pv0[:], V[N_KB - 1][:, h, :],
                                     prev_pt[:, 0:512],
                                     start=False, stop=True)
                    nc.tensor.matmul(pv1[:], V[N_KB - 1][:, h, :],
                                     prev_pt[:, 512:1024],
                                     start=False, stop=True)

                    for half, pv in ((0, pv0), (1, pv1)):
                        qs = slice((2 * qp + half) * 512,
                                   (2 * qp + half + 1) * 512)
                        den = smallp.tile([1, 512], F32, tag="den",
                                          name="den")
                        nc.vector.tensor_copy(den[:], pv[DH:DH + 1, :])
                        denb = smallp.tile([64, 512], F32, tag="denb",
                                           name="denb")
                        nc.gpsimd.partition_broadcast(denb[:], den[:])
                        rec = smallp.tile([64, 512], F32, tag="rec",
                                          name="rec")
                        nc.vector.reciprocal_approx_fast(rec[:], denb[:])
                        nc.vector.tensor_tensor(
                            CT[hp][rows, qs], pv[0:DH, :], rec[:], MULT)

                # output projection + chunked ReduceScatter
                for qb in (2 * qp, 2 * qp + 1):
                    for sb in range(4 * qb, 4 * qb + 4):
                        ys = scps.tile([128, 1024], F32, tag="sc", name="ys")
                        for eo in range(2):
                            for c in range(4):
                                nc.tensor.matmul(
                                    ys[:, eo * 512:(eo + 1) * 512],
                                    CT[c][:, sb * 128:(sb + 1) * 128],
                                    woT_t[c][:, eo * 512:(eo + 1) * 512],
                                    start=(c == 0), stop=(c == 3))
                        yt = youtp.tile([128, E], F32, tag="yt", name="yt")
                        nc.vector.tensor_add(yt[:], ys[:], bob[:])
                        nc.sync.dma_start(
                            y_part[sb * 128:(sb + 1) * 128, :], yt[:])
                    for i, (r0, n, after) in enumerate(RS_CHUNKS):
                        if after != qb:
                            continue
                        nc.gpsimd.collective_compute(
                            "ReduceScatter", mybir.AluOpType.add,
                            replica_groups=[[0, 1], [2, 3], [4, 5], [6, 7]],
                            ins=[y_part[r0:r0 + n, :]],
                            outs=[y_chunks[i][:]])
                        nc.sync.dma_start(
                            y_out[r0 // 2:(r0 + n) // 2, :], y_chunks[i][:])


def _get_nc():
    if "nc" not in _CACHE:
        _CACHE["nc"] = _build()
    return _CACHE["nc"]


def _make_in_maps(x, wq, wk, wv, wo, bo):
    bf16 = ml_dtypes.bfloat16
    x, wq, wk, wv, wo, bo = (np.asarray(a) for a in (x, wq, wk, wv, wo, bo))
    in_maps = []
    for c in range(N_CORES):
        b, g = c // TP, c % TP
        h0 = g * H_LOC
        xT_l = np.ascontiguousarray(x[b].T).astype(bf16)
        wq_l = np.ascontiguousarray(
            wq[h0:h0 + H_LOC].transpose(1, 0, 2).reshape(E, EI_LOC)).astype(bf16)
        wk_l = np.ascontiguousarray(
            wk[h0:h0 + H_LOC].transpose(1, 0, 2).reshape(E, EI_LOC)).astype(bf16)
        wv_l = np.ascontiguousarray(
            wv[h0:h0 + H_LOC].transpose(1, 0, 2).reshape(E, EI_LOC)).astype(bf16)
        woT_l = np.ascontiguousarray(
            wo[:, g * EI_LOC:(g + 1) * EI_LOC].T).astype(bf16)
        bob = np.broadcast_to(bo.astype(np.float32) / TP, (128, E)).copy()
        in_maps.append({
            "xT": xT_l, "wq": wq_l, "wk": wk_l, "wv": wv_l, "woT": woT_l,
            "bob": bob,
        })
    return in_maps


def _assemble(results):
    out = np.empty((B, S, E), dtype=np.float32)
    for c in range(N_CORES):
        b, g = c // TP, c % TP
        y = results[c]["y"]
        for r0, n, _ in RS_CHUNKS:
            half = n // 2
            out[b, r0 + g * half:r0 + (g + 1) * half, :] =                 y[r0 // 2:r0 // 2 + half, :]
    return out


def kernel(x, wq, wk, wv, wo, bo):
    nc = _get_nc()
    in_maps = _make_in_maps(x, wq, wk, wv, wo, bo)
    res = run_bass_kernel_spmd(nc, in_maps, list(range(N_CORES)))
    return _assemble(res.results)



# revision 8
# speedup vs baseline: 1.0077x; 1.0077x over previous
"""Multi-head attention Trainium2 kernel (B=4, S=2048, E=1024, H=16).

Sharding: 8 cores = 4 batch groups x 2-way head tensor-parallel.
Core c handles batch b=c//2 and heads [g*8, g*8+8) with g=c%2.
Partial output projections are pair-summed by chunked 2-way
ReduceScatters; the host assembles the full [4,2048,1024] result.

v2 layout: the kernel is scheduled around the Scalar (ACT) engine,
whose exp stream (33.5M elem/core) is the throughput floor (~294us):
- Loop nest is head-pair outer, 512-query strip inner. Scores for the
  two heads of a pair run CONCURRENTLY on the PE via 64x128 row tiling
  (tile_position (0,0)/(64,0)) since the contraction dim is DH=64.
  One exp instruction (N=1024) covers both heads' strips.
- Q/K/V/output projections are emitted as interleaved "units" inside
  the attention kb-loops so they execute in PE slack under the exp
  stream instead of serializing in front of it.
- Scores PSUM is double-buffered; PV accumulates into [65,512] banks
  (65th V column = softmax denominator).
- Output rows ReduceScatter in 5 chunks (512x3, 256x2) so the final
  collective+DMA tail is short.
"""

import os
import sys
from collections import deque

import numpy as np

for _p in ("/opt/trn_rl_repo", "/root/.axon_site/_ro/trn_rl_repo"):
    if os.path.isdir(_p) and _p not in sys.path:
        sys.path.append(_p)

import ml_dtypes  # noqa: E402
from concourse import bacc, mybir, tile  # noqa: E402
from concourse.bass_utils import run_bass_kernel_spmd  # noqa: E402

B, S, E, H, DH = 4, 2048, 1024, 16, 64
N_CORES = 8
TP = 2  # head-parallel factor within a batch
H_LOC = H // TP  # 8 heads per core
EI_LOC = H_LOC * DH  # 512 local rows of the concat dim
N_EC = E // 128  # 8 contraction chunks
N_KB = S // 128  # 16 key blocks
N_ST = S // 512  # 4 query strips
N_HP = H_LOC // 2  # 4 head pairs
# ReduceScatter chunks: (row0, nrows); emitted as soon as rows complete
RS_CHUNKS = [(0, 512), (512, 512), (1024, 512), (1536, 256), (1792, 256)]

BF = mybir.dt.bfloat16
F32 = mybir.dt.float32
EXP = mybir.ActivationFunctionType.Exp
MULT = mybir.AluOpType.mult

_CACHE = {}


def _build():
    nc = bacc.Bacc("TRN2", target_bir_lowering=False, debug=False,
                   num_devices=N_CORES)

    xT_in = nc.declare_dram_parameter("xT", [E, S], BF, isOutput=False)
    wq_in = nc.declare_dram_parameter("wq", [E, EI_LOC], BF, isOutput=False)
    wk_in = nc.declare_dram_parameter("wk", [E, EI_LOC], BF, isOutput=False)
    wv_in = nc.declare_dram_parameter("wv", [E, EI_LOC], BF, isOutput=False)
    woT_in = nc.declare_dram_parameter("woT", [EI_LOC, E], BF, isOutput=False)
    bob_in = nc.declare_dram_parameter("bob", [128, E], F32, isOutput=False)
    y_out = nc.declare_dram_parameter("y", [S // TP, E], F32, isOutput=True)

    y_part = nc.dram_tensor("y_part", [S, E], F32)
    y_chunks = [nc.dram_tensor(f"y_chunk{i}", [n // 2, E], F32)
                for i, (_, n) in enumerate(RS_CHUNKS)]

    inv_sqrt_dh = 1.0 / float(np.sqrt(DH))

    with tile.TileContext(nc) as tc:
        with (
            tc.tile_pool(name="const", bufs=1) as constp,
            tc.tile_pool(name="persist", bufs=1) as persist,
            tc.tile_pool(name="ptp", bufs=4) as ptp,
            tc.tile_pool(name="smallp", bufs=4) as smallp,
            tc.tile_pool(name="youtp", bufs=2) as youtp,
        ):
            # ---- input DMAs ----
            xT = [persist.tile([128, S], BF, tag=f"xT{ec}", name=f"xT{ec}")
                  for ec in range(N_EC)]
            wv_t, wk_t, wq_t = [], [], []
            for lst, src, nm in ((wv_t, wv_in, "wv"),):
                for ec in range(N_EC):
                    t = constp.tile([128, EI_LOC], BF, tag=f"{nm}{ec}",
                                    name=f"{nm}{ec}")
                    nc.sync.dma_start(t[:], src[ec * 128:(ec + 1) * 128, :])
                    lst.append(t)
            cs0 = slice(0, 512)
            for ec in range(N_EC):
                nc.sync.dma_start(
                    xT[ec][:, cs0], xT_in[ec * 128:(ec + 1) * 128, cs0])
            for lst, src, nm in ((wk_t, wk_in, "wk"), (wq_t, wq_in, "wq")):
                for ec in range(N_EC):
                    t = constp.tile([128, EI_LOC], BF, tag=f"{nm}{ec}",
                                    name=f"{nm}{ec}")
                    nc.sync.dma_start(t[:], src[ec * 128:(ec + 1) * 128, :])
                    lst.append(t)
            for q in range(1, 4):
                cs = slice(q * 512, (q + 1) * 512)
                for ec in range(N_EC):
                    nc.sync.dma_start(
                        xT[ec][:, cs], xT_in[ec * 128:(ec + 1) * 128, cs])
            woT_t = []
            for c in range(4):
                t = constp.tile([128, E], BF, tag=f"woT{c}", name=f"woT{c}")
                nc.sync.dma_start(t[:], woT_in[c * 128:(c + 1) * 128, :])
                woT_t.append(t)
            bob = constp.tile([128, E], F32, tag="bob")
            nc.sync.dma_start(bob[:], bob_in[:])

            # ---- persistent SBUF tensors ----
            # KT2 [hp]: rows 0-63 = head 2hp (d dim), 64-127 = head 2hp+1.
            # QTz [h]: per-head, head's d on the rows matching its KT2 slot
            # (even: 0-63, odd: 64-127), other 64 rows ZERO so a K=128
            # matmul against the paired KT2 yields that head's scores alone
            # (keeps every matmul in 128-row mode; no PE-tiling switches).
            QTz = [persist.tile([128, S], BF, tag=f"QT{h}", name=f"QT{h}")
                   for h in range(H_LOC)]
            KT2 = [persist.tile([128, S], BF, tag=f"KT{p}", name=f"KT{p}")
                   for p in range(N_HP)]
            for h in range(H_LOC):
                if h % 2 == 0:
                    nc.vector.memset(QTz[h][64:128, :], 0.0)
                else:
                    nc.vector.memset(QTz[h][0:64, :], 0.0)
            V = [persist.tile([128, H_LOC, DH + 1], BF, tag=f"V{s}",
                              name=f"V{s}") for s in range(N_KB)]
            CT = [persist.tile([128, S], BF, tag=f"CT{c}", name=f"CT{c}")
                  for c in range(4)]

            with (
                tc.tile_pool(name="projps", bufs=2, space="PSUM") as projps,
                tc.tile_pool(name="scps", bufs=2, space="PSUM") as scps,
                tc.tile_pool(name="pvps", bufs=2, space="PSUM") as pvps,
            ):
                def v_unit(sb):
                    ps = projps.tile([128, EI_LOC], F32, tag="proj",
                                     name="vps")
                    for ec in range(N_EC):
                        nc.tensor.matmul(
                            ps[:], xT[ec][:, sb * 128:(sb + 1) * 128],
                            wv_t[ec][:], start=(ec == 0),
                            stop=(ec == N_EC - 1))
                    nc.vector.tensor_copy(V[sb][:, :, 0:DH], ps[:])
                    nc.vector.memset(V[sb][:, :, DH], 1.0)

                def qk_unit(kind, hp, s):
                    w = wk_t if kind == "k" else wq_t
                    ps = projps.tile([128, EI_LOC], F32, tag="proj",
                                     name="qkps")
                    cs = slice(s * 512, (s + 1) * 512)
                    for ec in range(N_EC):
                        nc.tensor.matmul(
                            ps[:], w[ec][:, hp * 128:(hp + 1) * 128],
                            xT[ec][:, cs], start=(ec == 0),
                            stop=(ec == N_EC - 1))
                    if kind == "k":
                        nc.vector.tensor_copy(KT2[hp][:, cs], ps[:])
                    else:
                        nc.vector.tensor_copy(
                            QTz[2 * hp][0:64, cs], ps[0:64, :])
                        nc.vector.tensor_copy(
                            QTz[2 * hp + 1][64:128, cs], ps[64:128, :])

                # ---- pre-loop: V proj + full K(hp0) + Q(hp0, s0) ----
                for sb in range(4):
                    v_unit(sb)
                qk_unit("k", 0, 0)
                for sb in range(4, 8):
                    v_unit(sb)
                qk_unit("k", 0, 1)
                for sb in range(8, 10):
                    v_unit(sb)
                qk_unit("k", 0, 2)
                for sb in range(10, 12):
                    v_unit(sb)
                qk_unit("k", 0, 3)
                qk_unit("q", 0, 0)

                # deferred proj units, ordered by deadline; consumed inside
                # the attention kb-loops (<=5 per iteration)
                uq = deque()
                for sb in range(12, 16):
                    uq.append(("v", sb, 0))
                uq.append(("q", 0, 1))
                for u in (("q", 0, 2), ("k", 1, 0), ("k", 1, 1),
                          ("q", 0, 3), ("k", 1, 2), ("k", 1, 3),
                          ("q", 1, 0),
                          ("q", 1, 1), ("k", 2, 0),
                          ("q", 1, 2), ("k", 2, 1),
                          ("q", 1, 3), ("k", 2, 2), ("k", 2, 3),
                          ("q", 2, 0),
                          ("q", 2, 1), ("k", 3, 0),
                          ("q", 2, 2), ("k", 3, 1),
                          ("q", 2, 3), ("k", 3, 2), ("k", 3, 3),
                          ("q", 3, 0),
                          ("q", 3, 1), ("q", 3, 2), ("q", 3, 3)):
                    uq.append(u)
                # units allowed per attention iteration (16 iters)
                units_per_iter = [5, 3, 3, 1, 2, 2, 3, 1, 2, 2, 3, 1,
                                  1, 1, 1, 0]

                def emit_unit():
                    kind, a, b2 = uq.popleft()
                    if kind == "v":
                        v_unit(a)
                    else:
                        qk_unit(kind, a, b2)

                def outproj_strip(s):
                    for sb in range(4 * s, 4 * s + 4):
                        yt = youtp.tile([128, E], F32, tag="yt", name="yt")
                        for eo in range(2):
                            ys = projps.tile([128, EI_LOC], F32, tag="proj",
                                             name="ys")
                            for c in range(4):
                                nc.tensor.matmul(
                                    ys[:],
                                    CT[c][:, sb * 128:(sb + 1) * 128],
                                    woT_t[c][:, eo * 512:(eo + 1) * 512],
                                    start=(c == 0), stop=(c == 3))
                            nc.vector.tensor_add(
                                yt[:, eo * 512:(eo + 1) * 512], ys[:],
                                bob[:, eo * 512:(eo + 1) * 512])
                        nc.sync.dma_start(
                            y_part[sb * 128:(sb + 1) * 128, :], yt[:])
                        for i, (r0, n) in enumerate(RS_CHUNKS):
                            if r0 + n != (sb + 1) * 128:
                                continue
                            nc.gpsimd.collective_compute(
                                "ReduceScatter", mybir.AluOpType.add,
                                replica_groups=[[0, 1], [2, 3],
                                                [4, 5], [6, 7]],
                                ins=[y_part[r0:r0 + n, :]],
                                outs=[y_chunks[i][:]])
                            nc.sync.dma_start(
                                y_out[r0 // 2:(r0 + n) // 2, :],
                                y_chunks[i][:])

                # ---- main attention loop: head-pair outer, strip inner ----
                it = 0
                for hp in range(N_HP):
                    hA, hB = 2 * hp, 2 * hp + 1
                    for s in range(N_ST):
                        qsl = slice(s * 512, (s + 1) * 512)
                        nu = units_per_iter[it]
                        slots = {1 + 3 * j for j in range(nu)}
                        pvA = pvps.tile([DH + 1, 512], F32, tag="pv",
                                        name="pvA")
                        pvB = pvps.tile([DH + 1, 512], F32, tag="pv",
                                        name="pvB")
                        prev_pt = None
                        for kb in range(N_KB):
                            ksl = slice(kb * 128, (kb + 1) * 128)
                            sp = scps.tile([128, 1024], F32, tag="sc",
                                           name="sc")
                            nc.tensor.matmul(
                                sp[:, 0:512], KT2[hp][:, ksl],
                                QTz[hA][:, qsl])
                            nc.tensor.matmul(
                                sp[:, 512:1024], KT2[hp][:, ksl],
                                QTz[hB][:, qsl])
                            pt = ptp.tile([128, 1024], BF, tag="pt",
                                          name="pt")
                            nc.scalar.activation(pt[:], sp[:], EXP,
                                                 scale=inv_sqrt_dh)
                            if prev_pt is not None:
                                pkb = kb - 1
                                nc.tensor.matmul(
                                    pvA[:], V[pkb][:, hA, :],
                                    prev_pt[:, 0:512],
                                    start=(pkb == 0), stop=False)
                                nc.tensor.matmul(
                                    pvB[:], V[pkb][:, hB, :],
                                    prev_pt[:, 512:1024],
                                    start=(pkb == 0), stop=False)
                            if kb in slots and uq:
                                emit_unit()
                            prev_pt = pt
                        nc.tensor.matmul(pvA[:], V[N_KB - 1][:, hA, :],
                                         prev_pt[:, 0:512],
                                         start=False, stop=True)
                        nc.tensor.matmul(pvB[:], V[N_KB - 1][:, hB, :],
                                         prev_pt[:, 512:1024],
                                         start=False, stop=True)

                        for pv, r0 in ((pvA, 0), (pvB, 64)):
                            den = smallp.tile([1, 512], F32, tag="den",
                                              name="den")
                            nc.vector.tensor_copy(den[:], pv[DH:DH + 1, :])
                            denb = smallp.tile([64, 512], F32, tag="denb",
                                               name="denb")
                            nc.gpsimd.partition_broadcast(denb[:], den[:])
                            rec = smallp.tile([64, 512], F32, tag="rec",
                                              name="rec")
                            nc.vector.reciprocal_approx_fast(rec[:], denb[:])
                            nc.vector.tensor_tensor(
                                CT[hp][r0:r0 + 64, qsl], pv[0:DH, :],
                                rec[:], MULT)

                        if hp == N_HP - 1:
                            outproj_strip(s)
                        it += 1

    nc.finalize()
    return nc


def _get_nc():
    if "nc" not in _CACHE:
        _CACHE["nc"] = _build()
    return _CACHE["nc"]


def _make_in_maps(x, wq, wk, wv, wo, bo):
    bf16 = ml_dtypes.bfloat16
    x, wq, wk, wv, wo, bo = (np.asarray(a) for a in (x, wq, wk, wv, wo, bo))
    in_maps = []
    for c in range(N_CORES):
        b, g = c // TP, c % TP
        h0 = g * H_LOC
        xT_l = np.ascontiguousarray(x[b].T).astype(bf16)
        wq_l = np.ascontiguousarray(
            wq[h0:h0 + H_LOC].transpose(1, 0, 2).reshape(E, EI_LOC)).astype(bf16)
        wk_l = np.ascontiguousarray(
            wk[h0:h0 + H_LOC].transpose(1, 0, 2).reshape(E, EI_LOC)).astype(bf16)
        wv_l = np.ascontiguousarray(
            wv[h0:h0 + H_LOC].transpose(1, 0, 2).reshape(E, EI_LOC)).astype(bf16)
        woT_l = np.ascontiguousarray(
            wo[:, g * EI_LOC:(g + 1) * EI_LOC].T).astype(bf16)
        bob = np.broadcast_to(bo.astype(np.float32) / TP, (128, E)).copy()
        in_maps.append({
            "xT": xT_l, "wq": wq_l, "wk": wk_l, "wv": wv_l, "woT": woT_l,
            "bob": bob,
        })
    return in_maps


def _assemble(results):
    out = np.empty((B, S, E), dtype=np.float32)
    for c in range(N_CORES):
        b, g = c // TP, c % TP
        y = results[c]["y"]
        for r0, n in RS_CHUNKS:
            half = n // 2
            out[b, r0 + g * half:r0 + (g + 1) * half, :] = \
                y[r0 // 2:r0 // 2 + half, :]
    return out


def kernel(x, wq, wk, wv, wo, bo):
    nc = _get_nc()
    in_maps = _make_in_maps(x, wq, wk, wv, wo, bo)
    res = run_bass_kernel_spmd(nc, in_maps, list(range(N_CORES)))
    return _assemble(res.results)


# revision 14
# speedup vs baseline: 1.0613x; 1.0532x over previous
"""Multi-head attention Trainium2 kernel (B=4, S=2048, E=1024, H=16).

Sharding: 8 cores = 4 batch groups x 2-way head tensor-parallel.
Core c handles batch b=c//2 and heads [g*8, g*8+8) with g=c%2.
Partial output projections are pair-summed by chunked 2-way
ReduceScatters (bf16 wire format); the host assembles the full
[4,2048,1024] f32 result.

The kernel is scheduled around the Scalar (ACT) engine, whose exp
stream (33.5M elem/core) is the throughput floor (~276us):
- Loop nest is head-pair outer, 512-query strip inner. Scores for the
  two heads of a pair run CONCURRENTLY on the PE via 64x128 row tiling
  (tile_position (0,0)/(64,0)) since the contraction dim is DH=64.
  One exp instruction (N=1024) covers both heads' strips.
- Q/K/V projections are emitted as interleaved "units" inside the
  attention kb-loops so they execute in PE slack under the exp stream.
  K(hp0)/Q(hp0,s0) go first (gating the first scores); V projection
  trails behind the exp stream (PV tolerates lag via deep pt pool).
- V carries a ones-column so PV also emits the softmax denominator.
- Output rows ReduceScatter in 5 chunks (512x3, 256x2) in bf16,
  converted to f32 on-chip, so the collective tail stays short.
"""

import os
import sys

import numpy as np

for _p in ("/opt/trn_rl_repo", "/root/.axon_site/_ro/trn_rl_repo"):
    if os.path.isdir(_p) and _p not in sys.path:
        sys.path.append(_p)

import ml_dtypes  # noqa: E402
from concourse import bacc, mybir, tile  # noqa: E402
from concourse.bass_utils import run_bass_kernel_spmd  # noqa: E402

B, S, E, H, DH = 4, 2048, 1024, 16, 64
N_CORES = 8
TP = 2  # head-parallel factor within a batch
H_LOC = H // TP  # 8 heads per core
EI_LOC = H_LOC * DH  # 512 local rows of the concat dim
N_EC = E // 128  # 8 contraction chunks
N_KB = S // 128  # 16 key blocks
N_ST = S // 512  # 4 query strips
N_HP = H_LOC // 2  # 4 head pairs
# ReduceScatter chunks: (row0, nrows); emitted as soon as rows complete
RS_CHUNKS = [(0, 512), (512, 512), (1024, 512), (1536, 256), (1792, 256)]

BF = mybir.dt.bfloat16
F32 = mybir.dt.float32
EXP = mybir.ActivationFunctionType.Exp
MULT = mybir.AluOpType.mult

_CACHE = {}

# deferred projection-unit schedule: pre-loop list, then per-iteration
# {kb_slot: unit} maps for the 16 attention iterations (hp-outer:
# it = 4*hp + s). Every unit must be EMITTED before its first consumer
# (Tile deps only order later instructions after earlier ones).
# Deadlines: Q(0,s) by it=s; K(hp,*)+Q(hp,0) by it=4*hp; Q(hp,s) by
# it=4*hp+s. V(sb) feeds PV of it0; it0 runs PV four kb behind the exp
# stream so v6-v15 can be emitted inside it0 ahead of their PV reads.
PRE_UNITS = [("k", 0, 0), ("k", 0, 1), ("k", 0, 2), ("k", 0, 3),
             ("q", 0, 0), ("v", 0, 0), ("v", 1, 0), ("v", 2, 0),
             ("v", 3, 0), ("v", 4, 0), ("v", 5, 0)]
ITER_UNITS = [
    {1: ("q", 0, 1), 2: ("v", 6, 0), 4: ("v", 7, 0), 6: ("v", 8, 0),
     8: ("v", 9, 0), 10: ("v", 10, 0), 11: ("v", 11, 0),
     12: ("v", 12, 0), 13: ("v", 13, 0), 14: ("v", 14, 0),
     15: ("v", 15, 0)},                                # it0
    {1: ("q", 0, 2)},                                  # it1
    {1: ("q", 0, 3), 3: ("k", 1, 0), 5: ("k", 1, 1)},  # it2
    {1: ("k", 1, 2), 3: ("k", 1, 3), 5: ("q", 1, 0)},  # it3
    {1: ("q", 1, 1)},                                  # it4
    {1: ("q", 1, 2)},                                  # it5
    {1: ("q", 1, 3), 3: ("k", 2, 0)},                  # it6
    {1: ("k", 2, 1), 3: ("k", 2, 2), 5: ("k", 2, 3),
     7: ("q", 2, 0)},                                  # it7
    {1: ("q", 2, 1)},                                  # it8
    {1: ("q", 2, 2)},                                  # it9
    {1: ("q", 2, 3), 3: ("k", 3, 0)},                  # it10
    {1: ("k", 3, 1), 3: ("k", 3, 2), 5: ("k", 3, 3),
     7: ("q", 3, 0)},                                  # it11
    {1: ("q", 3, 1)},                                  # it12
    {1: ("q", 3, 2)},                                  # it13
    {1: ("q", 3, 3)},                                  # it14
    {},                                                # it15
]


def _build():
    nc = bacc.Bacc("TRN2", target_bir_lowering=False, debug=False,
                   num_devices=N_CORES)

    xT_in = nc.declare_dram_parameter("xT", [E, S], BF, isOutput=False)
    wq_in = nc.declare_dram_parameter("wq", [E, EI_LOC], BF, isOutput=False)
    wk_in = nc.declare_dram_parameter("wk", [E, EI_LOC], BF, isOutput=False)
    wv_in = nc.declare_dram_parameter("wv", [E, EI_LOC], BF, isOutput=False)
    woT_in = nc.declare_dram_parameter("woT", [EI_LOC, E], BF, isOutput=False)
    bob_in = nc.declare_dram_parameter("bob", [128, E], F32, isOutput=False)
    y_out = nc.declare_dram_parameter("y", [S // TP, E], F32, isOutput=True)

    y_part = nc.dram_tensor("y_part", [S, E], BF)
    y_chunks = [nc.dram_tensor(f"y_chunk{i}", [n // 2, E], BF)
                for i, (_, n) in enumerate(RS_CHUNKS)]

    inv_sqrt_dh = 1.0 / float(np.sqrt(DH))

    with tile.TileContext(nc) as tc:
        with (
            tc.tile_pool(name="const", bufs=1) as constp,
            tc.tile_pool(name="persist", bufs=1) as persist,
            tc.tile_pool(name="ptp", bufs=10) as ptp,
            tc.tile_pool(name="smallp", bufs=2) as smallp,
            tc.tile_pool(name="youtp", bufs=2) as youtp,
            tc.tile_pool(name="convp", bufs=2) as convp,
        ):
            # ---- input DMAs (wk/xT first: K proj gates first scores) ----
            xT = [persist.tile([128, S], BF, tag=f"xT{ec}", name=f"xT{ec}")
                  for ec in range(N_EC)]
            wv_t, wk_t, wq_t = [], [], []
            for ec in range(N_EC):
                t = constp.tile([128, EI_LOC], BF, tag=f"wk{ec}",
                                name=f"wk{ec}")
                nc.sync.dma_start(t[:], wk_in[ec * 128:(ec + 1) * 128, :])
                wk_t.append(t)
            for ec in range(N_EC):
                t = constp.tile([128, EI_LOC], BF, tag=f"wv{ec}",
                                name=f"wv{ec}")
                nc.sync.dma_start(t[:], wv_in[ec * 128:(ec + 1) * 128, :])
                wv_t.append(t)
            cs0 = slice(0, 512)
            for ec in range(N_EC):
                nc.sync.dma_start(
                    xT[ec][:, cs0], xT_in[ec * 128:(ec + 1) * 128, cs0])
            for ec in range(N_EC):
                t = constp.tile([128, EI_LOC], BF, tag=f"wq{ec}",
                                name=f"wq{ec}")
                nc.sync.dma_start(t[:], wq_in[ec * 128:(ec + 1) * 128, :])
                wq_t.append(t)
            for q in range(1, 4):
                cs = slice(q * 512, (q + 1) * 512)
                for ec in range(N_EC):
                    nc.sync.dma_start(
                        xT[ec][:, cs], xT_in[ec * 128:(ec + 1) * 128, cs])
            woT_t = []
            for c in range(4):
                t = constp.tile([128, E], BF, tag=f"woT{c}", name=f"woT{c}")
                nc.sync.dma_start(t[:], woT_in[c * 128:(c + 1) * 128, :])
                woT_t.append(t)
            bob = constp.tile([128, E], F32, tag="bob")
            nc.sync.dma_start(bob[:], bob_in[:])

            # ---- persistent SBUF tensors ----
            # QT2/KT2 [hp]: rows 0-63 = head 2hp (d dim), 64-127 = head 2hp+1
            QT2 = [persist.tile([128, S], BF, tag=f"QT{p}", name=f"QT{p}")
                   for p in range(N_HP)]
            KT2 = [persist.tile([128, S], BF, tag=f"KT{p}", name=f"KT{p}")
                   for p in range(N_HP)]
            V = [persist.tile([128, H_LOC, DH + 1], BF, tag=f"V{s}",
                              name=f"V{s}") for s in range(N_KB)]
            CT = [persist.tile([128, S], BF, tag=f"CT{c}", name=f"CT{c}")
                  for c in range(4)]

            with (
                tc.tile_pool(name="projps", bufs=2, space="PSUM") as projps,
                tc.tile_pool(name="scps", bufs=2, space="PSUM") as scps,
                tc.tile_pool(name="pvps", bufs=2, space="PSUM") as pvps,
            ):
                def v_unit(sb):
                    ps = projps.tile([128, EI_LOC], F32, tag="proj",
                                     name="vps")
                    for ec in range(N_EC):
                        nc.tensor.matmul(
                            ps[:], xT[ec][:, sb * 128:(sb + 1) * 128],
                            wv_t[ec][:], start=(ec == 0),
                            stop=(ec == N_EC - 1))
                    nc.vector.tensor_copy(V[sb][:, :, 0:DH], ps[:])
                    nc.vector.memset(V[sb][:, :, DH], 1.0)

                def qk_unit(kind, hp, s):
                    w = wk_t if kind == "k" else wq_t
                    dst = KT2 if kind == "k" else QT2
                    ps = projps.tile([128, EI_LOC], F32, tag="proj",
                                     name="qkps")
                    cs = slice(s * 512, (s + 1) * 512)
                    for ec in range(N_EC):
                        nc.tensor.matmul(
                            ps[:], w[ec][:, hp * 128:(hp + 1) * 128],
                            xT[ec][:, cs], start=(ec == 0),
                            stop=(ec == N_EC - 1))
                    nc.vector.tensor_copy(dst[hp][:, cs], ps[:])

                def emit_unit(u):
                    kind, a, b2 = u
                    if kind == "v":
                        v_unit(a)
                    else:
                        qk_unit(kind, a, b2)

                def outproj_unit(sb):
                    yt = youtp.tile([128, E], BF, tag="yt", name="yt")
                    for eo in range(2):
                        ys = projps.tile([128, EI_LOC], F32, tag="proj",
                                         name="ys")
                        for c in range(4):
                            nc.tensor.matmul(
                                ys[:],
                                CT[c][:, sb * 128:(sb + 1) * 128],
                                woT_t[c][:, eo * 512:(eo + 1) * 512],
                                start=(c == 0), stop=(c == 3))
                        nc.vector.tensor_add(
                            yt[:, eo * 512:(eo + 1) * 512], ys[:],
                            bob[:, eo * 512:(eo + 1) * 512])
                    nc.sync.dma_start(
                        y_part[sb * 128:(sb + 1) * 128, :], yt[:])
                    for i, (r0, n) in enumerate(RS_CHUNKS):
                        if r0 + n != (sb + 1) * 128:
                            continue
                        nc.gpsimd.collective_compute(
                            "ReduceScatter", mybir.AluOpType.add,
                            replica_groups=[[0, 1], [2, 3],
                                            [4, 5], [6, 7]],
                            ins=[y_part[r0:r0 + n, :]],
                            outs=[y_chunks[i][:]])
                        # bf16 chunk -> f32 output rows (via SBUF)
                        for rb in range(n // 256):
                            rows = slice(rb * 128, (rb + 1) * 128)
                            yb = convp.tile([128, E], BF, tag="yb",
                                            name="yb")
                            nc.sync.dma_start(yb[:], y_chunks[i][rows, :])
                            yf = convp.tile([128, E], F32, tag="yf",
                                            name="yf")
                            nc.vector.tensor_copy(yf[:], yb[:])
                            orow = r0 // 2 + rb * 128
                            nc.sync.dma_start(
                                y_out[orow:orow + 128, :], yf[:])

                for u in PRE_UNITS:
                    emit_unit(u)

                # ---- main attention loop: head-pair outer, strip inner ----
                # outproj for strip s is interleaved into iteration 13+s
                # (or appended after the loop for s3).
                it = 0
                for hp in range(N_HP):
                    hA, hB = 2 * hp, 2 * hp + 1
                    for s in range(N_ST):
                        qsl = slice(s * 512, (s + 1) * 512)
                        slots = dict(ITER_UNITS[it])
                        if 13 <= it <= 15:
                            for j in range(4):
                                slots[6 + 2 * j] = ("o", 4 * (it - 13) + j, 0)
                        pv_delay = 4 if it == 0 else 1
                        pvA = pvps.tile([DH + 1, 512], F32, tag="pv",
                                        name="pvA")
                        pvB = pvps.tile([DH + 1, 512], F32, tag="pv",
                                        name="pvB")
                        pts = {}
                        for kb in range(N_KB):
                            ksl = slice(kb * 128, (kb + 1) * 128)
                            sp = scps.tile([128, 1024], F32, tag="sc",
                                           name="sc")
                            nc.tensor.matmul(
                                sp[:, 0:512], KT2[hp][0:64, ksl],
                                QT2[hp][0:64, qsl], tile_position=(0, 0))
                            nc.tensor.matmul(
                                sp[:, 512:1024], KT2[hp][64:128, ksl],
                                QT2[hp][64:128, qsl], tile_position=(64, 0))
                            pt = ptp.tile([128, 1024], BF, tag="pt",
                                          name="pt")
                            nc.scalar.activation(pt[:], sp[:], EXP,
                                                 scale=inv_sqrt_dh)
                            pts[kb] = pt
                            u = slots.get(kb)
                            if u is not None:
                                if u[0] == "o":
                                    outproj_unit(u[1])
                                else:
                                    emit_unit(u)
                            pkb = kb - pv_delay
                            if pkb >= 0:
                                nc.tensor.matmul(
                                    pvA[:], V[pkb][:, hA, :],
                                    pts[pkb][:, 0:512],
                                    start=(pkb == 0), stop=False)
                                nc.tensor.matmul(
                                    pvB[:], V[pkb][:, hB, :],
                                    pts[pkb][:, 512:1024],
                                    start=(pkb == 0), stop=False)
                                del pts[pkb]
                        for pkb in range(N_KB - pv_delay, N_KB):
                            nc.tensor.matmul(
                                pvA[:], V[pkb][:, hA, :],
                                pts[pkb][:, 0:512],
                                start=False, stop=(pkb == N_KB - 1))
                            nc.tensor.matmul(
                                pvB[:], V[pkb][:, hB, :],
                                pts[pkb][:, 512:1024],
                                start=False, stop=(pkb == N_KB - 1))

                        for pv, r0 in ((pvA, 0), (pvB, 64)):
                            den = smallp.tile([1, 512], F32, tag="den",
                                              name="den")
                            nc.vector.tensor_copy(den[:], pv[DH:DH + 1, :])
                            denb = smallp.tile([64, 512], F32, tag="denb",
                                               name="denb")
                            nc.gpsimd.partition_broadcast(denb[:], den[:])
                            rec = smallp.tile([64, 512], F32, tag="rec",
                                              name="rec")
                            nc.vector.reciprocal_approx_fast(rec[:], denb[:])
                            nc.vector.tensor_tensor(
                                CT[hp][r0:r0 + 64, qsl], pv[0:DH, :],
                                rec[:], MULT)
                        it += 1
                for sb in range(12, 16):
                    outproj_unit(sb)

    nc.finalize()
    return nc


def _get_nc():
    if "nc" not in _CACHE:
        _CACHE["nc"] = _build()
    return _CACHE["nc"]


def _make_in_maps(x, wq, wk, wv, wo, bo):
    bf16 = ml_dtypes.bfloat16
    x, wq, wk, wv, wo, bo = (np.asarray(a) for a in (x, wq, wk, wv, wo, bo))
    in_maps = []
    for c in range(N_CORES):
        b, g = c // TP, c % TP
        h0 = g * H_LOC
        xT_l = np.ascontiguousarray(x[b].T).astype(bf16)
        wq_l = np.ascontiguousarray(
            wq[h0:h0 + H_LOC].transpose(1, 0, 2).reshape(E, EI_LOC)).astype(bf16)
        wk_l = np.ascontiguousarray(
            wk[h0:h0 + H_LOC].transpose(1, 0, 2).reshape(E, EI_LOC)).astype(bf16)
        wv_l = np.ascontiguousarray(
            wv[h0:h0 + H_LOC].transpose(1, 0, 2).reshape(E, EI_LOC)).astype(bf16)
        woT_l = np.ascontiguousarray(
            wo[:, g * EI_LOC:(g + 1) * EI_LOC].T).astype(bf16)
        bob = np.broadcast_to(bo.astype(np.float32) / TP, (128, E)).copy()
        in_maps.append({
            "xT": xT_l, "wq": wq_l, "wk": wk_l, "wv": wv_l, "woT": woT_l,
            "bob": bob,
        })
    return in_maps


def _assemble(results):
    out = np.empty((B, S, E), dtype=np.float32)
    for c in range(N_CORES):
        b, g = c // TP, c % TP
        y = results[c]["y"]
        for r0, n in RS_CHUNKS:
            half = n // 2
            out[b, r0 + g * half:r0 + (g + 1) * half, :] = \
                y[r0 // 2:r0 // 2 + half, :]
    return out


def kernel(x, wq, wk, wv, wo, bo):
    nc = _get_nc()
    in_maps = _make_in_maps(x, wq, wk, wv, wo, bo)
    res = run_bass_kernel_spmd(nc, in_maps, list(range(N_CORES)))
    return _assemble(res.results)


# revision 18
# speedup vs baseline: 1.0816x; 1.0191x over previous
"""Multi-head attention Trainium2 kernel (B=4, S=2048, E=1024, H=16).

Sharding: 8 cores = 4 batch groups x 2-way head tensor-parallel.
Core c handles batch b=c//2 and heads [g*8, g*8+8) with g=c%2.
Partial output projections are pair-summed by chunked 2-way
ReduceScatters (bf16 wire format); the host assembles the full
[4,2048,1024] f32 result.

The kernel is scheduled around the Scalar (ACT) engine, whose exp
stream (33.5M elem/core) is the throughput floor (~276us):
- Loop nest is head-pair outer, 512-query strip inner. Scores for the
  two heads of a pair run CONCURRENTLY on the PE via 64x128 row tiling
  (tile_position (0,0)/(64,0)) since the contraction dim is DH=64.
  One exp instruction (N=1024) covers both heads' strips.
- Q/K/V projections are emitted as interleaved "units" inside the
  attention kb-loops so they execute in PE slack under the exp stream.
  K(hp0)/Q(hp0,s0) go first (gating the first scores); V projection
  trails behind the exp stream (PV tolerates lag via deep pt pool).
- V carries a ones-column so PV also emits the softmax denominator.
- Output rows ReduceScatter in 5 chunks (512x3, 256x2) in bf16,
  converted to f32 on-chip, so the collective tail stays short.
"""

import os
import sys

import numpy as np

for _p in ("/opt/trn_rl_repo", "/root/.axon_site/_ro/trn_rl_repo"):
    if os.path.isdir(_p) and _p not in sys.path:
        sys.path.append(_p)

import ml_dtypes  # noqa: E402
from concourse import bacc, mybir, tile  # noqa: E402
from concourse.bass_utils import run_bass_kernel_spmd  # noqa: E402

B, S, E, H, DH = 4, 2048, 1024, 16, 64
N_CORES = 8
TP = 2  # head-parallel factor within a batch
H_LOC = H // TP  # 8 heads per core
EI_LOC = H_LOC * DH  # 512 local rows of the concat dim
N_EC = E // 128  # 8 contraction chunks
N_KB = S // 128  # 16 key blocks
N_ST = S // 512  # 4 query strips
N_HP = H_LOC // 2  # 4 head pairs
# ReduceScatter chunks: (row0, nrows); emitted as soon as rows complete
# (last chunk kept small so the end-of-kernel collective tail is short)
RS_CHUNKS = [(0, 512), (512, 512), (1024, 512), (1536, 384), (1920, 128)]

BF = mybir.dt.bfloat16
F32 = mybir.dt.float32
EXP = mybir.ActivationFunctionType.Exp
MULT = mybir.AluOpType.mult

_CACHE = {}

# deferred projection-unit schedule: pre-loop list, then per-iteration
# {kb_slot: unit} maps for the 16 attention iterations (hp-outer:
# it = 4*hp + s). Every unit must be EMITTED before its first consumer
# (Tile deps only order later instructions after earlier ones).
# Deadlines: Q(0,s) by it=s; K(hp,*)+Q(hp,0) by it=4*hp; Q(hp,s) by
# it=4*hp+s. V(sb) feeds PV of it0; it0 runs PV four kb behind the exp
# stream so v6-v15 can be emitted inside it0 ahead of their PV reads.
PRE_UNITS = [("k", 0, 0), ("k", 0, 1), ("k", 0, 2), ("k", 0, 3),
             ("q", 0, 0), ("v", 0, 0), ("v", 1, 0), ("v", 2, 0),
             ("v", 3, 0), ("v", 4, 0), ("v", 5, 0)]
ITER_UNITS = [
    {1: ("q", 0, 1), 2: ("v", 6, 0), 4: ("v", 7, 0), 6: ("v", 8, 0),
     8: ("v", 9, 0), 10: ("v", 10, 0), 11: ("v", 11, 0),
     12: ("v", 12, 0), 13: ("v", 13, 0), 14: ("v", 14, 0),
     15: ("v", 15, 0)},                                # it0
    {1: ("q", 0, 2)},                                  # it1
    {1: ("q", 0, 3), 3: ("k", 1, 0), 5: ("k", 1, 1)},  # it2
    {1: ("k", 1, 2), 3: ("k", 1, 3), 5: ("q", 1, 0)},  # it3
    {1: ("q", 1, 1)},                                  # it4
    {1: ("q", 1, 2)},                                  # it5
    {1: ("q", 1, 3), 3: ("k", 2, 0)},                  # it6
    {1: ("k", 2, 1), 3: ("k", 2, 2), 5: ("k", 2, 3),
     7: ("q", 2, 0)},                                  # it7
    {1: ("q", 2, 1)},                                  # it8
    {1: ("q", 2, 2)},                                  # it9
    {1: ("q", 2, 3), 3: ("k", 3, 0)},                  # it10
    {1: ("k", 3, 1), 3: ("k", 3, 2), 5: ("k", 3, 3),
     7: ("q", 3, 0)},                                  # it11
    {1: ("q", 3, 1)},                                  # it12
    {1: ("q", 3, 2)},                                  # it13
    {1: ("q", 3, 3)},                                  # it14
    {},                                                # it15
]


def _build():
    nc = bacc.Bacc("TRN2", target_bir_lowering=False, debug=False,
                   num_devices=N_CORES)

    xT_in = nc.declare_dram_parameter("xT", [E, S], BF, isOutput=False)
    wq_in = nc.declare_dram_parameter("wq", [E, EI_LOC], BF, isOutput=False)
    wk_in = nc.declare_dram_parameter("wk", [E, EI_LOC], BF, isOutput=False)
    wv_in = nc.declare_dram_parameter("wv", [E, EI_LOC], BF, isOutput=False)
    woT_in = nc.declare_dram_parameter("woT", [EI_LOC, E], BF, isOutput=False)
    bob_in = nc.declare_dram_parameter("bob", [128, E], F32, isOutput=False)
    y_out = nc.declare_dram_parameter("y", [S // TP, E], F32, isOutput=True)

    y_part = nc.dram_tensor("y_part", [S, E], BF)
    y_chunks = [nc.dram_tensor(f"y_chunk{i}", [n // 2, E], BF)
                for i, (_, n) in enumerate(RS_CHUNKS)]

    inv_sqrt_dh = 1.0 / float(np.sqrt(DH))

    with tile.TileContext(nc) as tc:
        with (
            tc.tile_pool(name="const", bufs=1) as constp,
            tc.tile_pool(name="persist", bufs=1) as persist,
            tc.tile_pool(name="ptp", bufs=10) as ptp,
            tc.tile_pool(name="smallp", bufs=2) as smallp,
            tc.tile_pool(name="youtp", bufs=2) as youtp,
            tc.tile_pool(name="convp", bufs=2) as convp,
        ):
            # ---- input DMAs (contiguous full tiles; wk/xT first since the
            # K projection gates the first scores) ----
            xT = [persist.tile([128, S], BF, tag=f"xT{ec}", name=f"xT{ec}")
                  for ec in range(N_EC)]
            wv_t, wk_t, wq_t = [], [], []
            for ec in range(N_EC):
                t = constp.tile([128, EI_LOC], BF, tag=f"wk{ec}",
                                name=f"wk{ec}")
                nc.sync.dma_start(t[:], wk_in[ec * 128:(ec + 1) * 128, :])
                wk_t.append(t)
            for ec in range(N_EC):
                nc.sync.dma_start(xT[ec][:], xT_in[ec * 128:(ec + 1) * 128, :])
            for ec in range(N_EC):
                t = constp.tile([128, EI_LOC], BF, tag=f"wv{ec}",
                                name=f"wv{ec}")
                nc.sync.dma_start(t[:], wv_in[ec * 128:(ec + 1) * 128, :])
                wv_t.append(t)
            for ec in range(N_EC):
                t = constp.tile([128, EI_LOC], BF, tag=f"wq{ec}",
                                name=f"wq{ec}")
                nc.sync.dma_start(t[:], wq_in[ec * 128:(ec + 1) * 128, :])
                wq_t.append(t)
            woT_t = []
            for c in range(4):
                t = constp.tile([128, E], BF, tag=f"woT{c}", name=f"woT{c}")
                nc.sync.dma_start(t[:], woT_in[c * 128:(c + 1) * 128, :])
                woT_t.append(t)
            bob = constp.tile([128, E], F32, tag="bob")
            nc.sync.dma_start(bob[:], bob_in[:])

            # ---- persistent SBUF tensors ----
            # QT2/KT2 [hp]: rows 0-63 = head 2hp (d dim), 64-127 = head 2hp+1
            QT2 = [persist.tile([128, S], BF, tag=f"QT{p}", name=f"QT{p}")
                   for p in range(N_HP)]
            KT2 = [persist.tile([128, S], BF, tag=f"KT{p}", name=f"KT{p}")
                   for p in range(N_HP)]
            V = [persist.tile([128, H_LOC, DH + 1], BF, tag=f"V{s}",
                              name=f"V{s}") for s in range(N_KB)]
            CT = [persist.tile([128, S], BF, tag=f"CT{c}", name=f"CT{c}")
                  for c in range(4)]

            with (
                tc.tile_pool(name="projps", bufs=2, space="PSUM") as projps,
                tc.tile_pool(name="scps", bufs=2, space="PSUM") as scps,
                tc.tile_pool(name="pvps", bufs=2, space="PSUM") as pvps,
            ):
                def v_unit(sb):
                    ps = projps.tile([128, EI_LOC], F32, tag="proj",
                                     name="vps")
                    for ec in range(N_EC):
                        nc.tensor.matmul(
                            ps[:], xT[ec][:, sb * 128:(sb + 1) * 128],
                            wv_t[ec][:], start=(ec == 0),
                            stop=(ec == N_EC - 1))
                    nc.vector.tensor_copy(V[sb][:, :, 0:DH], ps[:])
                    nc.vector.memset(V[sb][:, :, DH], 1.0)

                def qk_unit(kind, hp, s):
                    w = wk_t if kind == "k" else wq_t
                    dst = KT2 if kind == "k" else QT2
                    ps = projps.tile([128, EI_LOC], F32, tag="proj",
                                     name="qkps")
                    cs = slice(s * 512, (s + 1) * 512)
                    for ec in range(N_EC):
                        nc.tensor.matmul(
                            ps[:], w[ec][:, hp * 128:(hp + 1) * 128],
                            xT[ec][:, cs], start=(ec == 0),
                            stop=(ec == N_EC - 1))
                    nc.vector.tensor_copy(dst[hp][:, cs], ps[:])

                def emit_unit(u):
                    kind, a, b2 = u
                    if kind == "v":
                        v_unit(a)
                    else:
                        qk_unit(kind, a, b2)

                def outproj_unit(sb):
                    yt = youtp.tile([128, E], BF, tag="yt", name="yt")
                    for eo in range(2):
                        ys = projps.tile([128, EI_LOC], F32, tag="proj",
                                         name="ys")
                        for c in range(4):
                            nc.tensor.matmul(
                                ys[:],
                                CT[c][:, sb * 128:(sb + 1) * 128],
                                woT_t[c][:, eo * 512:(eo + 1) * 512],
                                start=(c == 0), stop=(c == 3))
                        nc.vector.tensor_add(
                            yt[:, eo * 512:(eo + 1) * 512], ys[:],
                            bob[:, eo * 512:(eo + 1) * 512])
                    nc.sync.dma_start(
                        y_part[sb * 128:(sb + 1) * 128, :], yt[:])
                    for i, (r0, n) in enumerate(RS_CHUNKS):
                        if r0 + n != (sb + 1) * 128:
                            continue
                        nc.gpsimd.collective_compute(
                            "ReduceScatter", mybir.AluOpType.add,
                            replica_groups=[[0, 1], [2, 3],
                                            [4, 5], [6, 7]],
                            ins=[y_part[r0:r0 + n, :]],
                            outs=[y_chunks[i][:]])
                        # bf16 chunk -> f32 output rows (via SBUF)
                        r = 0
                        while r < n // 2:
                            rr = min(128, n // 2 - r)
                            yb = convp.tile([128, E], BF, tag="yb",
                                            name="yb")
                            nc.sync.dma_start(
                                yb[0:rr, :], y_chunks[i][r:r + rr, :])
                            yf = convp.tile([128, E], F32, tag="yf",
                                            name="yf")
                            nc.vector.tensor_copy(yf[0:rr, :], yb[0:rr, :])
                            orow = r0 // 2 + r
                            nc.sync.dma_start(
                                y_out[orow:orow + rr, :], yf[0:rr, :])
                            r += rr

                for u in PRE_UNITS:
                    emit_unit(u)

                # ---- main attention loop: head-pair outer, strip inner ----
                # outproj for strip s is interleaved into iteration 13+s
                # (or appended after the loop for s3).
                it = 0
                for hp in range(N_HP):
                    hA, hB = 2 * hp, 2 * hp + 1
                    for s in range(N_ST):
                        qsl = slice(s * 512, (s + 1) * 512)
                        slots = dict(ITER_UNITS[it])
                        if 13 <= it <= 15:
                            for j in range(4):
                                slots[2 + 2 * j] = ("o", 4 * (it - 13) + j, 0)
                        pv_delay = 4 if it == 0 else 2
                        pvA = pvps.tile([DH + 1, 512], F32, tag="pv",
                                        name="pvA")
                        pvB = pvps.tile([DH + 1, 512], F32, tag="pv",
                                        name="pvB")
                        pts = {}

                        def pv_pair(pkb):
                            nc.tensor.matmul(
                                pvA[:], V[pkb][:, hA, :], pts[pkb][:, 0:512],
                                start=(pkb == 0), stop=(pkb == N_KB - 1))
                            nc.tensor.matmul(
                                pvB[:], V[pkb][:, hB, :],
                                pts[pkb][:, 512:1024],
                                start=(pkb == 0), stop=(pkb == N_KB - 1))
                            del pts[pkb]

                        # 2-kb blocks: 4 scores MMs (64-row mode) then 4 PV
                        # MMs (128-row mode) so PE tiling-mode switches occur
                        # once per direction per block, not per kb.
                        for kb2 in range(0, N_KB, 2):
                            for kb in (kb2, kb2 + 1):
                                ksl = slice(kb * 128, (kb + 1) * 128)
                                sp = scps.tile([128, 1024], F32, tag="sc",
                                               name="sc")
                                nc.tensor.matmul(
                                    sp[:, 0:512], KT2[hp][0:64, ksl],
                                    QT2[hp][0:64, qsl], tile_position=(0, 0))
                                nc.tensor.matmul(
                                    sp[:, 512:1024], KT2[hp][64:128, ksl],
                                    QT2[hp][64:128, qsl],
                                    tile_position=(64, 0))
                                pt = ptp.tile([128, 1024], BF, tag="pt",
                                              name="pt")
                                nc.scalar.activation(pt[:], sp[:], EXP,
                                                     scale=inv_sqrt_dh)
                                pts[kb] = pt
                            for kb in (kb2, kb2 + 1):
                                u = slots.get(kb)
                                if u is not None:
                                    if u[0] == "o":
                                        outproj_unit(u[1])
                                    else:
                                        emit_unit(u)
                            for kb in (kb2, kb2 + 1):
                                pkb = kb - pv_delay
                                if pkb >= 0:
                                    pv_pair(pkb)
                        for pkb in range(N_KB - pv_delay, N_KB):
                            pv_pair(pkb)

                        for pv, r0 in ((pvA, 0), (pvB, 64)):
                            den = smallp.tile([1, 512], F32, tag="den",
                                              name="den")
                            nc.vector.tensor_copy(den[:], pv[DH:DH + 1, :])
                            denb = smallp.tile([64, 512], F32, tag="denb",
                                               name="denb")
                            nc.gpsimd.partition_broadcast(denb[:], den[:])
                            rec = smallp.tile([64, 512], F32, tag="rec",
                                              name="rec")
                            nc.vector.reciprocal_approx_fast(rec[:], denb[:])
                            nc.vector.tensor_tensor(
                                CT[hp][r0:r0 + 64, qsl], pv[0:DH, :],
                                rec[:], MULT)
                        it += 1
                for sb in range(12, 16):
                    outproj_unit(sb)

    nc.finalize()
    return nc


def _get_nc():
    if "nc" not in _CACHE:
        _CACHE["nc"] = _build()
    return _CACHE["nc"]


def _make_in_maps(x, wq, wk, wv, wo, bo):
    bf16 = ml_dtypes.bfloat16
    x, wq, wk, wv, wo, bo = (np.asarray(a) for a in (x, wq, wk, wv, wo, bo))
    in_maps = []
    for c in range(N_CORES):
        b, g = c // TP, c % TP
        h0 = g * H_LOC
        xT_l = np.ascontiguousarray(x[b].T).astype(bf16)
        wq_l = np.ascontiguousarray(
            wq[h0:h0 + H_LOC].transpose(1, 0, 2).reshape(E, EI_LOC)).astype(bf16)
        wk_l = np.ascontiguousarray(
            wk[h0:h0 + H_LOC].transpose(1, 0, 2).reshape(E, EI_LOC)).astype(bf16)
        wv_l = np.ascontiguousarray(
            wv[h0:h0 + H_LOC].transpose(1, 0, 2).reshape(E, EI_LOC)).astype(bf16)
        woT_l = np.ascontiguousarray(
            wo[:, g * EI_LOC:(g + 1) * EI_LOC].T).astype(bf16)
        bob = np.broadcast_to(bo.astype(np.float32) / TP, (128, E)).copy()
        in_maps.append({
            "xT": xT_l, "wq": wq_l, "wk": wk_l, "wv": wv_l, "woT": woT_l,
            "bob": bob,
        })
    return in_maps


def _assemble(results):
    out = np.empty((B, S, E), dtype=np.float32)
    for c in range(N_CORES):
        b, g = c // TP, c % TP
        y = results[c]["y"]
        for r0, n in RS_CHUNKS:
            half = n // 2
            out[b, r0 + g * half:r0 + (g + 1) * half, :] = \
                y[r0 // 2:r0 // 2 + half, :]
    return out


def kernel(x, wq, wk, wv, wo, bo):
    nc = _get_nc()
    in_maps = _make_in_maps(x, wq, wk, wv, wo, bo)
    res = run_bass_kernel_spmd(nc, in_maps, list(range(N_CORES)))
    return _assemble(res.results)
